# revision 7
# baseline (speedup 1.0000x reference)
"""Binary conv + BN(train) + ReLU fused Trainium2 SPMD kernel.

Reference computation (NCHW, x:(32,256,56,56) f32):
    mean/var over (N,H,W) per channel; xn = (x-mean)*rsqrt(var+eps)*gamma+beta
    xb = sign(xn); wb = sign(W); y = relu(conv3x3(xb, wb, pad=1) + bias)

Strategy: data-parallel over batch across 8 NeuronCores (4 images each).
Per-core partial BN stats (bn_stats/bn_aggr on DVE, pipelined with the x
load) are combined with a 2KB AllReduce; normalize+sign runs as one
scalar-engine activation (Sign(a*x+b)) writing fp8 into zero-padded 58x58
planes; the 3x3 conv is 9 accumulating DoubleRow fp8 matmuls (K=256 via
the paired-row mode) per 128x448 output tile; bias+relu is fused into the
PSUM->SBUF drain on ScalarE. Sign values are exact in fp8/bf16 and PSUM
accumulates in fp32, so the binarized conv is exact.
"""

import sys
import time

for _p in ("/opt/trn_rl_repo", "/root/.axon_site/_ro/trn_rl_repo"):
    if _p not in sys.path:
        sys.path.append(_p)

import numpy as np

import concourse.bass as bass
from concourse.bass import ds
import concourse.mybir as mybir
import concourse.tile as tile
from concourse import bacc, bass_utils

F32 = mybir.dt.float32
BF16 = mybir.dt.bfloat16
FP8 = mybir.dt.float8e4
U32 = mybir.dt.uint32
AF = mybir.ActivationFunctionType

N_CORES = 8
NB = 4          # images per core
C = 256
P = 128         # partitions / chunk size
NCH = 2         # channel chunks (ci and co)
H = W = 56
HW = H * W      # 3136
PH = PW = 58    # padded plane
PSZ = PH * PW   # 3364
RG = 8          # output rows per psum tile
NG = H // RG    # 7 row groups
NT = RG * W     # 448 columns per matmul
BN_EPS = 1e-5
BLK = 8         # psum tiles in flight per weight-reuse block

USE_FP8 = True
ELIDE_LDW = True  # skip redundant weight reloads within a weight-reuse block
PAIR_SWAP = False  # pair-swap variant measured slower (SWDGE critical-section latency)

_CACHE = {}


def _build_nc():
    act_dt = FP8 if USE_FP8 else BF16
    nc = bacc.Bacc("TRN2", target_bir_lowering=False, debug=False,
                   num_devices=N_CORES)
    xs = nc.dram_tensor("xs", [NB, C, H, W], F32, kind="ExternalInput")
    if USE_FP8:
        wt = nc.dram_tensor("wt", [P, NCH, 9 * NCH * P], FP8, kind="ExternalInput")
    else:
        wt = nc.dram_tensor("wt", [NCH, P, 9, NCH, P], BF16, kind="ExternalInput")
    par = nc.dram_tensor("par", [NCH, P, 3], F32, kind="ExternalInput")
    ys = nc.dram_tensor("ys", [NB, C, H, W], F32, kind="ExternalOutput")
    if PAIR_SWAP:
        nonce = nc.dram_tensor("nonce", [1, 16], U32, kind="ExternalInput")
        # pair-scoped shared scratch ("Shared" DRAM aliases per NC pair): used
        # to swap the two strided-group sums between pair mates
        ps_sh = nc.dram_tensor("ps_sh", [2, P, 2 * NCH], F32, kind="Internal",
                               addr_space="Shared")
        pf_sh = nc.dram_tensor("pf_sh", [2, 16], U32, kind="Internal",
                               addr_space="Shared")

    with tile.TileContext(nc) as tc:
        with (
            tc.tile_pool(name="main", bufs=1) as main,
            tc.tile_pool(name="outp", bufs=4) as outp,
            tc.tile_pool(name="psum", bufs=8, space="PSUM") as psum,
            tc.tile_pool(name="dram", bufs=1, space="DRAM") as dram,
        ):
            xt = [main.tile([P, NB * HW], F32, name=f"xt{c}") for c in range(NCH)]
            # sign planes: [p, ci_chunk, image, padded 58x58] (chunk dim = fp8
            # DoubleRow pair dim)
            xball = main.tile([P, NCH, NB * PSZ], act_dt, name="xball")
            xbv = xball.rearrange("p j (n h w) -> p j n h w", n=NB, h=PH)
            if USE_FP8:
                wb = main.tile([P, NCH, 9 * NCH * P], FP8, name="wb")
            else:
                wb = main.tile([P, NCH, 9 * NCH * P], BF16, name="wb")
            parc = main.tile([P, 3 * NCH], F32, name="parc")  # [gamma,beta,bias] x chunk
            st6 = [main.tile([P, NB * 7 * 6], F32, name=f"st6{c}") for c in range(NCH)]

            # load x (channels on partitions) + one-pass partial stats,
            # pipelined per (image, chunk); issue from ScalarE (HWDGE) which
            # starts earlier than Sync in the Tile preamble
            for n in range(NB):
                eng = nc.scalar if n < 2 else nc.sync
                for c in range(NCH):
                    eng.dma_start(
                        xt[c][:, n * HW:(n + 1) * HW],
                        xs[n, c * P:(c + 1) * P].rearrange("p h w -> p (h w)"),
                    )
            if USE_FP8:
                nc.sync.dma_start(wb[:], wt[:])
            else:
                nc.sync.dma_start(
                    wb[:],
                    wt.rearrange("c p t o m -> p c (t o m)"),
                )
            nc.sync.dma_start(
                parc.rearrange("p (c s) -> p c s", s=3),
                par.rearrange("c p s -> p c s"),
            )

            # zero only the pad borders of the sign planes (GpSimd; interior
            # is fully overwritten by the Sign activation)
            for c in range(NCH):
                bval = 0.5 if c == 1 else 0.0
                for n in range(NB):
                    nc.gpsimd.memset(xbv[:, c, n, 0, :], bval)
                    nc.gpsimd.memset(xbv[:, c, n, PH - 1, :], bval)
                    nc.gpsimd.memset(xbv[:, c, n, 1:PH - 1, 0], bval)
                    nc.gpsimd.memset(xbv[:, c, n, 1:PH - 1, PW - 1], bval)

            for n in (0, 2, 1, 3):
                for c in range(NCH):
                    for g in range(7):
                        nc.vector.bn_stats(
                            st6[c][:, (n * 7 + g) * 6:(n * 7 + g + 1) * 6],
                            xt[c][:, n * HW + g * NT: n * HW + (g + 1) * NT],
                        )

            # per-core (mean, var) -> (mean/8, E[x^2]/8) for the all-reduce
            mv = main.tile([P, 2 * NCH], F32)
            pre = main.tile([P, 2 * NCH], F32)
            t_a = main.tile([P, 1], F32)
            t_b = main.tile([P, 1], F32)
            for c in range(NCH):
                nc.vector.bn_aggr(mv[:, 2 * c:2 * c + 2], st6[c][:])
                mean = mv[:, 2 * c:2 * c + 1]
                var = mv[:, 2 * c + 1:2 * c + 2]
                nc.vector.tensor_mul(t_a[:], mean, mean)
                nc.vector.tensor_add(t_b[:], var, t_a[:])
                nc.vector.tensor_scalar_mul(pre[:, 2 * c:2 * c + 1], mean, 1.0 / N_CORES)
                nc.vector.tensor_scalar_mul(pre[:, 2 * c + 1:2 * c + 2], t_b[:], 1.0 / N_CORES)

            # strided 4-rank AllReduce (one rank per pair) is much cheaper
            # than the 8-rank one; the missing half-sum lives on the pair
            # mate and is swapped through pair-shared DRAM below
            cc_in = dram.tile([P, 2 * NCH], F32)
            cc_out = dram.tile([P, 2 * NCH], F32)
            nc.sync.dma_start(cc_in[:], pre[:])
            if not PAIR_SWAP:
                nc.gpsimd.collective_compute(
                    "AllReduce",
                    mybir.AluOpType.add,
                    replica_groups=[list(range(N_CORES))],
                    ins=[cc_in[:].opt()],
                    outs=[cc_out[:].opt()],
                )
                gs = main.tile([P, 2 * NCH], F32)
                nc.sync.dma_start(gs[:], cc_out[:])
            else:
                nt = main.tile([1, 16], U32, name="nt")
                nc.sync.dma_start(nt[:], nonce[:])
                nc.gpsimd.collective_compute(
                    "AllReduce",
                    mybir.AluOpType.add,
                    replica_groups=[[0, 2, 4, 6], [1, 3, 5, 7]],
                    ins=[cc_in[:].opt()],
                    outs=[cc_out[:].opt()],
                )
                gsb = main.tile([P, 2 * NCH], F32, name="gsb")
                goth = main.tile([P, 2, 2 * NCH], F32, name="goth")
                pb = main.tile([1, 16], U32, name="pb")
                with tc.tile_critical():
                    eng = nc.gpsimd
                    pid = eng.partition_id()
                    slot = pid % 2
                    sem = nc.alloc_semaphore("pair_sem")
                    psem = nc.alloc_semaphore("poll_sem")
                    eng.dma_start(gsb[:], cc_out[:]).then_inc(sem, 16)
                    eng.wait_ge(sem, 16)
                    eng.dma_start(
                        ps_sh[ds(slot, 1)].rearrange("o p s -> (o p) s"), gsb[:]
                    ).then_inc(sem, 16)
                    eng.wait_ge(sem, 32)
                    eng.dma_start(
                        pf_sh[ds(slot, 1)].rearrange("o s -> (o s)")[None, :], nt[:]
                    ).then_inc(sem, 16)
                    eng.wait_ge(sem, 48)
                    # bounded spin: re-DMA each pair flag to SBUF per iteration
                    # until it carries this run's nonce (reg_load straight from
                    # DRAM does not observe remote DMA writes)
                    nreg = eng.alloc_register("nreg")
                    eng.reg_load(nreg, nt[0:1, 0:1])
                    nval = eng.snap(nreg, donate=True)
                    cnt = eng.alloc_register("cnt")
                    eng.reg_mov(cnt, 0)
                    it_reg = eng.alloc_register("it")
                    eng.reg_mov(it_reg, 0)
                    for j in range(2):
                        def cond(j=j):
                            eng.reg_add(cnt, cnt, 16)
                            cv = eng.snap(cnt)
                            eng.dma_start(
                                pb[0:1, j:j + 1], pf_sh[j:j + 1, 0:1]
                            ).then_inc(psem, 16)
                            eng.wait_ge(psem, cv)
                            r = eng.alloc_register(f"pr{j}")
                            eng.reg_load(r, pb[0:1, j:j + 1])
                            eng.reg_add(it_reg, it_reg, 1)
                            rv = eng.snap(r, donate=True)
                            itv = eng.snap(it_reg)
                            return (rv != nval) & (itv < 3000)
                        with eng.While(cond):
                            pass
                    eng.dma_start(
                        goth[:], ps_sh.rearrange("o p s -> p o s")
                    ).then_inc(sem, 16)
                    eng.wait_ge(sem, 64)
                gs = main.tile([P, 2 * NCH], F32)
                nc.vector.tensor_add(gs[:], goth[:, 0, :], goth[:, 1, :])

            # a = gamma*rsqrt(var+eps), b = beta - mean*a, both chunks at once
            # layouts: gs = [m0,e0,m1,e1]; ab = [a0,a1,b0,b1]
            ab = main.tile([P, 2 * NCH], F32)
            u1 = main.tile([P, NCH], F32)
            u2 = main.tile([P, NCH], F32)
            gsv = gs.rearrange("p (c s) -> p c s", s=2)
            gmean = gsv[:, :, 0]
            ex2 = gsv[:, :, 1]
            parv = parc.rearrange("p (c s) -> p c s", s=3)
            av = ab[:, 0:NCH]
            bv = ab[:, NCH:2 * NCH]
            nc.vector.tensor_mul(u1[:], gmean, gmean)
            nc.vector.tensor_sub(u2[:], ex2, u1[:])          # global var
            nc.vector.tensor_scalar_add(u2[:], u2[:], BN_EPS)
            nc.scalar.activation(u1[:], u2[:], AF.Sqrt)
            nc.vector.reciprocal(u2[:], u1[:])               # rsqrt
            nc.vector.tensor_mul(av, parv[:, :, 0], u2[:])
            nc.vector.tensor_mul(u1[:], gmean, av)
            nc.vector.tensor_sub(bv, parv[:, :, 1], u1[:])
            # chunk-1 threshold for the DVE is_gt path: t1 = -b1/a1 (a1 > 0)
            t1 = main.tile([P, 1], F32)
            nc.vector.reciprocal(t1[:], ab[:, 1:2])
            nc.vector.tensor_mul(t1[:], t1[:], ab[:, NCH + 1:NCH + 2])
            nc.vector.tensor_scalar_mul(t1[:], t1[:], -1.0)

            # normalize + sign -> padded planes; split rows so the first conv
            # block (rows 0..33 of image 0) unblocks as early as possible
            # chunk0 as +-1 on ScalarE; chunk1 as {0,1} = [x > t1] on the
            # otherwise-idle DVE (host doubles chunk-1 weights and folds the
            # constant correction sum(w_c1) into the drain bias; pad borders
            # are 0.5 so 2*0.5*w == w cancels exactly in the correction)
            for n in range(NB):
                for c in range(NCH):
                    slices = ((0, 18), (18, 34), (34, H)) if n == 0 else ((0, 34), (34, H))
                    for r0, r1 in slices:
                        dst = xbv[:, c, n, 1 + r0:1 + r1, 1:1 + W]
                        srcv = (xt[c][:, n * HW + r0 * W:n * HW + r1 * W]
                                .rearrange("p (h w) -> p h w", w=W))
                        if c == 0:
                            nc.scalar.activation(
                                dst, srcv, AF.Sign,
                                bias=ab[:, NCH:NCH + 1],
                                scale=ab[:, 0:1],
                            )
                        else:
                            nc.vector.tensor_scalar(
                                dst, srcv, t1[:], None, mybir.AluOpType.is_gt,
                            )

            # 3x3 binary conv; first block is small so matmuls start right
            # after the first sign rows land
            jobs = [(n, g) for n in range(NB) for g in range(NG)]
            blocks = []
            # image-aligned blocks for o=0: a block never needs signs of a
            # later image than its predecessors (the 6.3us mid-block stall
            # seen when a block spanned the image-0/1 boundary)
            steps = {0: [2, 5, 7, 7, 7], 1: [8, 8, 8, 2, 2]}
            for o in range(NCH):
                pos = 0
                for step in steps[o]:
                    blocks.append((o, jobs[pos:pos + step]))
                    pos += step
                assert pos == len(jobs)
            for o, blk in blocks:
                if True:
                    pts = [psum.tile([P, NT], F32, name="ps", tag="ps") for _ in blk]
                    if USE_FP8:
                        for t in range(9):
                            ky, kx = divmod(t, 3)
                            w_ap = wb[:, :, (t * NCH + o) * P:(t * NCH + o + 1) * P]
                            for k, (n, g) in enumerate(blk):
                                rhs = xbv[:, :, n, g * RG + ky: g * RG + ky + RG, kx:kx + W]
                                mm = nc.tensor.matmul(
                                    pts[k][:], w_ap, rhs,
                                    start=(t == 0), stop=(t == 8),
                                    perf_mode=mybir.MatmulPerfMode.DoubleRow,
                                )
                                if ELIDE_LDW and k > 0:
                                    mm.ins.ldweights = False
                    else:
                        for c in range(NCH):
                            for t in range(9):
                                ky, kx = divmod(t, 3)
                                w_ap = wb[:, c, (t * NCH + o) * P:(t * NCH + o + 1) * P]
                                first = (c == 0 and t == 0)
                                last = (c == NCH - 1 and t == 8)
                                for k, (n, g) in enumerate(blk):
                                    rhs = xbv[:, c, n, g * RG + ky: g * RG + ky + RG, kx:kx + W]
                                    mm = nc.tensor.matmul(pts[k][:], w_ap, rhs,
                                                          start=first, stop=last)
                                    if ELIDE_LDW and k > 0:
                                        mm.ins.ldweights = False
                    for k, (n, g) in enumerate(blk):
                        ob = outp.tile([P, NT], F32, name="ob", tag="ob")
                        nc.scalar.activation(ob[:], pts[k][:], AF.Relu,
                                             bias=parc[:, 3 * o + 2:3 * o + 3])
                        nc.sync.dma_start(
                            ys[n, o * P:(o + 1) * P, g * RG:(g + 1) * RG, :],
                            ob.rearrange("p (h w) -> p h w", w=W),
                        )
    nc.compile()
    return nc


def _get_nc():
    if "nc" not in _CACHE:
        _CACHE["nc"] = _build_nc()
    return _CACHE["nc"]


def _prep_inputs(x, gamma, beta, weight, bias):
    wsign = np.sign(weight.astype(np.float32))
    if USE_FP8:
        # [p(ci_in), j(ci_chunk), (tap, o_chunk, co_in)]; chunk-1 input rows
        # carry 2w because their activations are encoded as {0,1} not +-1
        wstack = (
            wsign.reshape(NCH, P, NCH, P, 3, 3)      # o, m, c, p, ky, kx
            .transpose(3, 2, 4, 5, 0, 1)             # p, c, ky, kx, o, m
            .copy()
        )
        wstack[:, 1] *= 2.0
        wT = wstack.reshape(P, NCH, 9 * NCH * P).astype(mybir.dt.np(FP8))
    else:
        wT = (
            wsign.reshape(NCH, P, NCH, P, 3, 3)      # o, m, c, p, ky, kx
            .transpose(2, 3, 4, 5, 0, 1)             # c, p, ky, kx, o, m
            .reshape(NCH, P, 9, NCH, P)
            .astype(mybir.dt.np(BF16))
        )
    k_o = wsign[:, P:, :, :].sum(axis=(1, 2, 3)).astype(np.float32)
    par = np.stack(
        [gamma.astype(np.float32), beta.astype(np.float32),
         bias.astype(np.float32) - k_o],
        axis=-1,
    ).reshape(NCH, P, 3)
    x = np.ascontiguousarray(x, dtype=np.float32)
    in_maps = [
        {"xs": x[j * NB:(j + 1) * NB], "wt": wT, "par": par}
        for j in range(N_CORES)
    ]
    if PAIR_SWAP:
        # fresh per-call nonce so pair-flag state from a previous execution of
        # the same loaded NEFF can never satisfy this run's barrier
        _CACHE["nonce_ctr"] = _CACHE.get("nonce_ctr", 0) + 1
        seed = (int(time.time() * 1e6) ^ (_CACHE["nonce_ctr"] * 0x9E3779B1)) & 0x7FFFFFFF
        nonce = np.full((1, 16), np.uint32(seed | 1), dtype=np.uint32)
        for m in in_maps:
            m["nonce"] = nonce
    return in_maps


def _run(x, gamma, beta, weight, bias, trace=False):
    nc = _get_nc()
    in_maps = _prep_inputs(x, gamma, beta, weight, bias)
    res = bass_utils.run_bass_kernel_spmd(
        nc, in_maps, core_ids=list(range(N_CORES)), trace=trace
    )
    out = np.concatenate([res.results[j]["ys"] for j in range(N_CORES)], axis=0)
    return out, res


def kernel(x, gamma, beta, weight, bias):
    out, _ = _run(x, gamma, beta, weight, bias, trace=False)
    return out



# revision 9
# speedup vs baseline: 1.2338x; 1.2338x over previous
"""Binary conv + BN(train) + ReLU fused Trainium2 SPMD kernel.

Reference computation (NCHW, x:(32,256,56,56) f32):
    mean/var over (N,H,W) per channel; xn = (x-mean)*rsqrt(var+eps)*gamma+beta
    xb = sign(xn); wb = sign(W); y = relu(conv3x3(xb, wb, pad=1) + bias)

Strategy: data-parallel over batch across 8 NeuronCores (4 images each).
Per-core partial BN stats (bn_stats/bn_aggr on DVE, pipelined with the x
load) are combined with a 2KB AllReduce; normalize+sign runs as one
scalar-engine activation (Sign(a*x+b)) writing fp8 into zero-padded 58x58
planes; the 3x3 conv is 9 accumulating DoubleRow fp8 matmuls (K=256 via
the paired-row mode) per 128x448 output tile; bias+relu is fused into the
PSUM->SBUF drain on ScalarE. Sign values are exact in fp8/bf16 and PSUM
accumulates in fp32, so the binarized conv is exact.
"""

import sys
import time

for _p in ("/opt/trn_rl_repo", "/root/.axon_site/_ro/trn_rl_repo"):
    if _p not in sys.path:
        sys.path.append(_p)

import numpy as np

import concourse.bass as bass
from concourse.bass import ds
import concourse.mybir as mybir
import concourse.tile as tile
from concourse import bacc, bass_utils

F32 = mybir.dt.float32
BF16 = mybir.dt.bfloat16
FP8 = mybir.dt.float8e4
U32 = mybir.dt.uint32
AF = mybir.ActivationFunctionType

N_CORES = 8
NB = 4          # images per core
C = 256
P = 128         # partitions / chunk size
NCH = 2         # channel chunks (ci and co)
H = W = 56
HW = H * W      # 3136
PH = PW = 58    # padded plane
PSZ = PH * PW   # 3364
RG = 8          # output rows per psum tile
NG = H // RG    # 7 row groups
NT = RG * W     # 448 columns per matmul
BN_EPS = 1e-5
BLK = 8         # psum tiles in flight per weight-reuse block

USE_FP8 = True
ELIDE_LDW = True  # skip redundant weight reloads within a weight-reuse block
PAIR_SWAP = False  # pair-swap variant measured slower (SWDGE critical-section latency)

_CACHE = {}


def _build_nc():
    act_dt = FP8 if USE_FP8 else BF16
    nc = bacc.Bacc("TRN2", target_bir_lowering=False, debug=False,
                   num_devices=N_CORES)
    xs = nc.dram_tensor("xs", [NB, C, H, W], F32, kind="ExternalInput")
    if USE_FP8:
        wt = nc.dram_tensor("wt", [P, NCH, 9 * NCH * P], FP8, kind="ExternalInput")
    else:
        wt = nc.dram_tensor("wt", [NCH, P, 9, NCH, P], BF16, kind="ExternalInput")
    par = nc.dram_tensor("par", [NCH, P, 3], F32, kind="ExternalInput")
    ys = nc.dram_tensor("ys", [NB, C, H, W], F32, kind="ExternalOutput")
    if PAIR_SWAP:
        nonce = nc.dram_tensor("nonce", [1, 16], U32, kind="ExternalInput")
        # pair-scoped shared scratch ("Shared" DRAM aliases per NC pair): used
        # to swap the two strided-group sums between pair mates
        ps_sh = nc.dram_tensor("ps_sh", [2, P, 2 * NCH], F32, kind="Internal",
                               addr_space="Shared")
        pf_sh = nc.dram_tensor("pf_sh", [2, 16], U32, kind="Internal",
                               addr_space="Shared")

    with tile.TileContext(nc) as tc:
        with (
            tc.tile_pool(name="main", bufs=1) as main,
            tc.tile_pool(name="outp", bufs=4) as outp,
            tc.tile_pool(name="psum", bufs=8, space="PSUM") as psum,
            tc.tile_pool(name="dram", bufs=1, space="DRAM") as dram,
        ):
            xt = [main.tile([P, NB * HW], F32, name=f"xt{c}") for c in range(NCH)]
            # sign planes: [p, ci_chunk, image, padded 58x58] (chunk dim = fp8
            # DoubleRow pair dim)
            xball = main.tile([P, NCH, NB * PSZ], act_dt, name="xball")
            xbv = xball.rearrange("p j (n h w) -> p j n h w", n=NB, h=PH)
            if USE_FP8:
                wb = main.tile([P, NCH, 9 * NCH * P], FP8, name="wb")
            else:
                wb = main.tile([P, NCH, 9 * NCH * P], BF16, name="wb")
            parc = main.tile([P, 3 * NCH], F32, name="parc")  # [gamma,beta,bias] x chunk
            st6 = [main.tile([P, NB * 7 * 6], F32, name=f"st6{c}") for c in range(NCH)]

            # load x (channels on partitions) + one-pass partial stats,
            # pipelined per (image, chunk); issue from ScalarE (HWDGE) which
            # starts earlier than Sync in the Tile preamble
            for n in range(NB):
                eng = nc.scalar if n < 2 else nc.sync
                for c in range(NCH):
                    eng.dma_start(
                        xt[c][:, n * HW:(n + 1) * HW],
                        xs[n, c * P:(c + 1) * P].rearrange("p h w -> p (h w)"),
                    )
            if USE_FP8:
                nc.sync.dma_start(wb[:], wt[:])
            else:
                nc.sync.dma_start(
                    wb[:],
                    wt.rearrange("c p t o m -> p c (t o m)"),
                )
            nc.sync.dma_start(
                parc.rearrange("p (c s) -> p c s", s=3),
                par.rearrange("c p s -> p c s"),
            )

            # zero only the pad borders of the sign planes (GpSimd; interior
            # is fully overwritten by the Sign activation)
            for c in range(NCH):
                bval = 0.5 if c == 1 else 0.0
                for n in range(NB):
                    nc.gpsimd.memset(xbv[:, c, n, 0, :], bval)
                    nc.gpsimd.memset(xbv[:, c, n, PH - 1, :], bval)
                    nc.gpsimd.memset(xbv[:, c, n, 1:PH - 1, 0], bval)
                    nc.gpsimd.memset(xbv[:, c, n, 1:PH - 1, PW - 1], bval)

            for n in (0, 2, 1, 3):
                for c in range(NCH):
                    for g in range(7):
                        nc.vector.bn_stats(
                            st6[c][:, (n * 7 + g) * 6:(n * 7 + g + 1) * 6],
                            xt[c][:, n * HW + g * NT: n * HW + (g + 1) * NT],
                        )

            # per-core (mean, var) -> (mean/8, E[x^2]/8) for the all-reduce
            mv = main.tile([P, 2 * NCH], F32)
            pre = main.tile([P, 2 * NCH], F32)
            t_a = main.tile([P, 1], F32)
            t_b = main.tile([P, 1], F32)
            for c in range(NCH):
                nc.vector.bn_aggr(mv[:, 2 * c:2 * c + 2], st6[c][:])
                mean = mv[:, 2 * c:2 * c + 1]
                var = mv[:, 2 * c + 1:2 * c + 2]
                nc.vector.tensor_mul(t_a[:], mean, mean)
                nc.vector.tensor_add(t_b[:], var, t_a[:])
                nc.vector.tensor_scalar_mul(pre[:, 2 * c:2 * c + 1], mean, 1.0 / N_CORES)
                nc.vector.tensor_scalar_mul(pre[:, 2 * c + 1:2 * c + 2], t_b[:], 1.0 / N_CORES)

            # strided 4-rank AllReduce (one rank per pair) is much cheaper
            # than the 8-rank one; the missing half-sum lives on the pair
            # mate and is swapped through pair-shared DRAM below
            cc_in = dram.tile([P, 2 * NCH], F32)
            cc_out = dram.tile([P, 2 * NCH], F32)
            nc.sync.dma_start(cc_in[:], pre[:])
            if not PAIR_SWAP:
                nc.gpsimd.collective_compute(
                    "AllReduce",
                    mybir.AluOpType.add,
                    replica_groups=[list(range(N_CORES))],
                    ins=[cc_in[:].opt()],
                    outs=[cc_out[:].opt()],
                )
                gs = main.tile([P, 2 * NCH], F32)
                nc.sync.dma_start(gs[:], cc_out[:])
            else:
                nt = main.tile([1, 16], U32, name="nt")
                nc.sync.dma_start(nt[:], nonce[:])
                nc.gpsimd.collective_compute(
                    "AllReduce",
                    mybir.AluOpType.add,
                    replica_groups=[[0, 2, 4, 6], [1, 3, 5, 7]],
                    ins=[cc_in[:].opt()],
                    outs=[cc_out[:].opt()],
                )
                gsb = main.tile([P, 2 * NCH], F32, name="gsb")
                goth = main.tile([P, 2, 2 * NCH], F32, name="goth")
                pb = main.tile([1, 16], U32, name="pb")
                with tc.tile_critical():
                    eng = nc.gpsimd
                    pid = eng.partition_id()
                    slot = pid % 2
                    sem = nc.alloc_semaphore("pair_sem")
                    psem = nc.alloc_semaphore("poll_sem")
                    eng.dma_start(gsb[:], cc_out[:]).then_inc(sem, 16)
                    eng.wait_ge(sem, 16)
                    eng.dma_start(
                        ps_sh[ds(slot, 1)].rearrange("o p s -> (o p) s"), gsb[:]
                    ).then_inc(sem, 16)
                    eng.wait_ge(sem, 32)
                    eng.dma_start(
                        pf_sh[ds(slot, 1)].rearrange("o s -> (o s)")[None, :], nt[:]
                    ).then_inc(sem, 16)
                    eng.wait_ge(sem, 48)
                    # bounded spin: re-DMA each pair flag to SBUF per iteration
                    # until it carries this run's nonce (reg_load straight from
                    # DRAM does not observe remote DMA writes)
                    nreg = eng.alloc_register("nreg")
                    eng.reg_load(nreg, nt[0:1, 0:1])
                    nval = eng.snap(nreg, donate=True)
                    cnt = eng.alloc_register("cnt")
                    eng.reg_mov(cnt, 0)
                    it_reg = eng.alloc_register("it")
                    eng.reg_mov(it_reg, 0)
                    for j in range(2):
                        def cond(j=j):
                            eng.reg_add(cnt, cnt, 16)
                            cv = eng.snap(cnt)
                            eng.dma_start(
                                pb[0:1, j:j + 1], pf_sh[j:j + 1, 0:1]
                            ).then_inc(psem, 16)
                            eng.wait_ge(psem, cv)
                            r = eng.alloc_register(f"pr{j}")
                            eng.reg_load(r, pb[0:1, j:j + 1])
                            eng.reg_add(it_reg, it_reg, 1)
                            rv = eng.snap(r, donate=True)
                            itv = eng.snap(it_reg)
                            return (rv != nval) & (itv < 3000)
                        with eng.While(cond):
                            pass
                    eng.dma_start(
                        goth[:], ps_sh.rearrange("o p s -> p o s")
                    ).then_inc(sem, 16)
                    eng.wait_ge(sem, 64)
                gs = main.tile([P, 2 * NCH], F32)
                nc.vector.tensor_add(gs[:], goth[:, 0, :], goth[:, 1, :])

            # a = gamma*rsqrt(var+eps), b = beta - mean*a, both chunks at once
            # layouts: gs = [m0,e0,m1,e1]; ab = [a0,a1,b0,b1]
            ab = main.tile([P, 2 * NCH], F32)
            u1 = main.tile([P, NCH], F32)
            u2 = main.tile([P, NCH], F32)
            gsv = gs.rearrange("p (c s) -> p c s", s=2)
            gmean = gsv[:, :, 0]
            ex2 = gsv[:, :, 1]
            parv = parc.rearrange("p (c s) -> p c s", s=3)
            av = ab[:, 0:NCH]
            bv = ab[:, NCH:2 * NCH]
            nc.vector.tensor_mul(u1[:], gmean, gmean)
            nc.vector.tensor_sub(u2[:], ex2, u1[:])          # global var
            nc.vector.tensor_scalar_add(u2[:], u2[:], BN_EPS)
            nc.scalar.activation(u1[:], u2[:], AF.Sqrt)
            nc.vector.reciprocal(u2[:], u1[:])               # rsqrt
            nc.vector.tensor_mul(av, parv[:, :, 0], u2[:])
            nc.vector.tensor_mul(u1[:], gmean, av)
            nc.vector.tensor_sub(bv, parv[:, :, 1], u1[:])
            # chunk-1 threshold for the DVE is_gt path: t1 = -b1/a1 (a1 > 0)
            t1 = main.tile([P, 1], F32)
            nc.vector.reciprocal(t1[:], ab[:, 1:2])
            nc.vector.tensor_mul(t1[:], t1[:], ab[:, NCH + 1:NCH + 2])
            nc.vector.tensor_scalar_mul(t1[:], t1[:], -1.0)

            # normalize + sign -> padded planes; split rows so the first conv
            # block (rows 0..33 of image 0) unblocks as early as possible
            # chunk0 as +-1 on ScalarE; chunk1 as {0,1} = [x > t1] on the
            # otherwise-idle DVE (host doubles chunk-1 weights and folds the
            # constant correction sum(w_c1) into the drain bias; pad borders
            # are 0.5 so 2w*0.5 == w cancels exactly in the correction)
            for n in range(NB):
                for c in range(NCH):
                    slices = ((0, 18), (18, 34), (34, H)) if n == 0 else ((0, 34), (34, H))
                    for r0, r1 in slices:
                        dst = xbv[:, c, n, 1 + r0:1 + r1, 1:1 + W]
                        srcv = (xt[c][:, n * HW + r0 * W:n * HW + r1 * W]
                                .rearrange("p (h w) -> p h w", w=W))
                        if c == 0:
                            nc.scalar.activation(
                                dst, srcv, AF.Sign,
                                bias=ab[:, NCH:NCH + 1],
                                scale=ab[:, 0:1],
                            )
                        else:
                            nc.vector.tensor_scalar(
                                dst, srcv, t1[:], None, mybir.AluOpType.is_gt,
                            )

            # 3x3 binary conv; first block is small so matmuls start right
            # after the first sign rows land
            jobs = [(n, g) for n in range(NB) for g in range(NG)]
            blocks = []
            # image-aligned blocks for o=0: a block never needs signs of a
            # later image than its predecessors (the 6.3us mid-block stall
            # seen when a block spanned the image-0/1 boundary)
            steps = {0: [2, 5, 7, 7, 7], 1: [8, 8, 8, 2, 2]}
            for o in range(NCH):
                pos = 0
                for step in steps[o]:
                    blocks.append((o, jobs[pos:pos + step]))
                    pos += step
                assert pos == len(jobs)
            for o, blk in blocks:
                if True:
                    pts = [psum.tile([P, NT], F32, name="ps", tag="ps") for _ in blk]
                    if USE_FP8:
                        for t in range(9):
                            ky, kx = divmod(t, 3)
                            w_ap = wb[:, :, (t * NCH + o) * P:(t * NCH + o + 1) * P]
                            for k, (n, g) in enumerate(blk):
                                rhs = xbv[:, :, n, g * RG + ky: g * RG + ky + RG, kx:kx + W]
                                mm = nc.tensor.matmul(
                                    pts[k][:], w_ap, rhs,
                                    start=(t == 0), stop=(t == 8),
                                    perf_mode=mybir.MatmulPerfMode.DoubleRow,
                                )
                                if ELIDE_LDW and k > 0:
                                    mm.ins.ldweights = False
                    else:
                        for c in range(NCH):
                            for t in range(9):
                                ky, kx = divmod(t, 3)
                                w_ap = wb[:, c, (t * NCH + o) * P:(t * NCH + o + 1) * P]
                                first = (c == 0 and t == 0)
                                last = (c == NCH - 1 and t == 8)
                                for k, (n, g) in enumerate(blk):
                                    rhs = xbv[:, c, n, g * RG + ky: g * RG + ky + RG, kx:kx + W]
                                    mm = nc.tensor.matmul(pts[k][:], w_ap, rhs,
                                                          start=first, stop=last)
                                    if ELIDE_LDW and k > 0:
                                        mm.ins.ldweights = False
                    for k, (n, g) in enumerate(blk):
                        ob = outp.tile([P, NT], F32, name="ob", tag="ob")
                        nc.scalar.activation(ob[:], pts[k][:], AF.Relu,
                                             bias=parc[:, 3 * o + 2:3 * o + 3])
                        nc.sync.dma_start(
                            ys[n, o * P:(o + 1) * P, g * RG:(g + 1) * RG, :],
                            ob.rearrange("p (h w) -> p h w", w=W),
                        )
    nc.compile()
    return nc


def _get_nc():
    if "nc" not in _CACHE:
        _CACHE["nc"] = _build_nc()
    return _CACHE["nc"]


def _prep_inputs(x, gamma, beta, weight, bias):
    wsign = np.sign(weight.astype(np.float32))
    if USE_FP8:
        # [p(ci_in), j(ci_chunk), (tap, o_chunk, co_in)]; chunk-1 input rows
        # carry 2w because their activations are encoded as {0,1} not +-1
        wstack = (
            wsign.reshape(NCH, P, NCH, P, 3, 3)      # o, m, c, p, ky, kx
            .transpose(3, 2, 4, 5, 0, 1)             # p, c, ky, kx, o, m
            .copy()
        )
        wstack[:, 1] *= 2.0
        wT = wstack.reshape(P, NCH, 9 * NCH * P).astype(mybir.dt.np(FP8))
    else:
        wT = (
            wsign.reshape(NCH, P, NCH, P, 3, 3)      # o, m, c, p, ky, kx
            .transpose(2, 3, 4, 5, 0, 1)             # c, p, ky, kx, o, m
            .reshape(NCH, P, 9, NCH, P)
            .astype(mybir.dt.np(BF16))
        )
    k_o = wsign[:, P:, :, :].sum(axis=(1, 2, 3)).astype(np.float32)
    par = np.stack(
        [gamma.astype(np.float32), beta.astype(np.float32),
         bias.astype(np.float32) - k_o],
        axis=-1,
    ).reshape(NCH, P, 3)
    x = np.ascontiguousarray(x, dtype=np.float32)
    in_maps = [
        {"xs": x[j * NB:(j + 1) * NB], "wt": wT, "par": par}
        for j in range(N_CORES)
    ]
    if PAIR_SWAP:
        # fresh per-call nonce so pair-flag state from a previous execution of
        # the same loaded NEFF can never satisfy this run's barrier
        _CACHE["nonce_ctr"] = _CACHE.get("nonce_ctr", 0) + 1
        seed = (int(time.time() * 1e6) ^ (_CACHE["nonce_ctr"] * 0x9E3779B1)) & 0x7FFFFFFF
        nonce = np.full((1, 16), np.uint32(seed | 1), dtype=np.uint32)
        for m in in_maps:
            m["nonce"] = nonce
    return in_maps


def _run(x, gamma, beta, weight, bias, trace=False):
    nc = _get_nc()
    in_maps = _prep_inputs(x, gamma, beta, weight, bias)
    res = bass_utils.run_bass_kernel_spmd(
        nc, in_maps, core_ids=list(range(N_CORES)), trace=trace
    )
    out = np.concatenate([res.results[j]["ys"] for j in range(N_CORES)], axis=0)
    return out, res


def kernel(x, gamma, beta, weight, bias):
    out, _ = _run(x, gamma, beta, weight, bias, trace=False)
    return out

